# revision 3
# baseline (speedup 1.0000x reference)
"""Trainium2 Bass kernel for sparse 3D voxel convolution (e3nn-style, 5^3 taps).

v2 design (vs baseline):
  - Only the 56 taps with non-negligible radial embedding are processed.
    The smooth_finite radial basis has cutoff 2.5, so 45 of 125 taps
    (|offset| > 2.5 and the center) have exactly-zero kernels; 24 more at
    d^2=6 carry ~1.2% of tap RMS weight and are dropped (adds ~2e-4 rel err).
  - bf16 feature table + transposed SWDGE gathers (dma_gather transpose=True)
    deliver X^T directly to SBUF: no PE transposes; PE runs bf16 (4x fp32).
  - Pair stream is tap-pure 128-token columns; gather ops span tap
    boundaries at 768 idx (the transpose-gather ucode cap); one bf16
    scatter-add per tap into 8 rotating partial tables (SWDGE queues 0-2).
  - Center tap kernel is exactly zero => center pass is just the residual
    e3nn Linear: 16-row-block transpose gathers (elem_size=2048) pinned to
    SWDGE queue 3 (mixing elem sizes on one queue corrupts), interleaved
    among the sparse taps so PE/HWDGE work hides under SWDGE descriptor gen.
  - Host sums out + tbl0..7 during unshard (commutative adds; avoids any
    device-side ordering between SWDGE scatter writes and readback).
"""

import os
import sys
import types

import numpy as np
import ml_dtypes

BF16 = ml_dtypes.bfloat16

NRB = 8
RAD = 2.5
GRID = 192
N = 200000
DIM = 80
EP = 128                      # padded feature row (bf16 -> 256B)
ALPHA = 1.0 / np.sqrt(48.0)
N_CORES = 8
N_LOC = N // N_CORES          # 25000 dst voxels per core
CEN_BLK = 2048
N_CEN = 13 * CEN_BLK          # 26624 center rows (covers N_LOC w/ padding)
NT = N_CEN                    # out/table rows
PAD_DST = 25088               # scatter pad destinations land in [PAD_DST, NT)
GMAX = int(os.environ.get("K_GMAX", "768"))  # transpose-gather idx cap (ucode fails at 1024)
SMAX = 1024                   # scatter idx cap per op
N_TBL = int(os.environ.get("K_TABLES", "8"))
TAP_EMB_THRESH = 0.05 if os.environ.get("K_D6", "0") != "1" else 1e-6

_ax = np.arange(-2.0, 3.0, dtype=np.float32)
LATTICE = np.stack(np.meshgrid(_ax, _ax, _ax, indexing="ij"), -1)
PERM = np.arange(125).reshape(5, 5, 5).transpose(2, 1, 0).reshape(-1)
OFFS = LATTICE.reshape(-1, 3).astype(np.int32)[PERM]
CENTER_TAP = 62


def _radial_emb():
    d = np.linalg.norm(LATTICE, axis=-1)
    centers = np.linspace(0.0, RAD, NRB + 2)[1:-1]
    step = centers[1] - centers[0]
    t = (d[..., None] - centers) / step
    inside = np.abs(t) < 1.0
    safe = np.where(inside, 1.0 - t * t, 1.0)
    return (1.14136 * np.exp(2.0) * np.where(inside, np.exp(-2.0 / safe), 0.0)).astype(
        np.float32
    )


EMB = _radial_emb().reshape(-1, NRB)[PERM]
TAPS = [
    t for t in range(125)
    if t != CENTER_TAP and np.abs(EMB[t]).max() > TAP_EMB_THRESH
]
NTAPS = len(TAPS)


def make_kernel_np(weight):
    w = (EMB @ weight.astype(np.float32)) / 125.0  # [125, 2304] (already PERM order)
    w1 = w[:, :1024].reshape(125, 32, 32)
    w2 = w[:, 1024:1536].reshape(125, 32, 16)
    w3 = w[:, 1536:1792].reshape(125, 16, 16)
    w4 = w[:, 1792:].reshape(125, 16, 32)
    s0 = SH[:, 0]
    v = SH[:, 1:]
    eye3 = np.eye(3, dtype=w.dtype)
    K00 = ALPHA * w1 * s0[:, None, None]
    K01 = ALPHA * np.einsum("pik,pm->pikm", w2, v).reshape(125, 32, 48)
    K11 = ALPHA * np.einsum(
        "pik,mn->pimkn", w3 * s0[:, None, None], eye3
    ).reshape(125, 48, 48)
    K10 = (ALPHA / np.sqrt(3.0)) * np.einsum("pik,pm->pimk", w4, v).reshape(125, 48, 32)
    return np.concatenate(
        [np.concatenate([K00, K01], 2), np.concatenate([K10, K11], 2)], 1
    )


def _sph():
    n = np.linalg.norm(LATTICE, axis=-1, keepdims=True)
    u = np.where(n > 0, LATTICE / np.maximum(n, 1e-9), 0.0)
    return np.concatenate([np.ones_like(n), np.sqrt(3.0) * u], -1).astype(np.float32)


SH = _sph().reshape(-1, 4)[PERM]


def w_sc_embed(w_sc0, w_sc1):
    W = np.zeros((80, 80), np.float32)
    W[:32, :32] = w_sc0 / np.sqrt(32.0)
    blk = np.zeros((48, 48), np.float32)
    for m in range(3):
        blk[m::3, m::3] = w_sc1 / np.sqrt(16.0)
    W[32:, 32:] = blk
    return W


def build_pairs(coords):
    idx_vol = np.full(GRID * GRID * GRID, -1, np.int32)
    lin = (coords[:, 0].astype(np.int64) * GRID + coords[:, 1]) * GRID + coords[:, 2]
    idx_vol[lin] = np.arange(N, dtype=np.int32)
    all_i = np.arange(N, dtype=np.int32)
    pairs = {}
    for t in TAPS:
        c = coords + OFFS[t]
        ok = np.all((c >= 0) & (c < GRID), axis=1)
        cl = (c[:, 0].astype(np.int64) * GRID + c[:, 1]) * GRID + c[:, 2]
        cl = np.clip(cl, 0, GRID**3 - 1)
        nb = idx_vol[cl]
        valid = ok & (nb >= 0)
        pairs[t] = (all_i[valid], nb[valid])
    return pairs


def wrap16(a):
    """Token stream [n] -> [128, n//16] int16 (16-partition wrap, 8x replicated)."""
    n = a.shape[0]
    w = a.reshape(n // 16, 16).T
    return np.ascontiguousarray(np.tile(w, (8, 1)).astype(np.int16))


def build_plan(feats, coords):
    order = np.argsort(coords[:, 0], kind="stable").astype(np.int32)
    pos = np.empty(N, np.int32)
    pos[order] = np.arange(N, dtype=np.int32)
    core_of = pos // N_LOC
    loc_dst = pos % N_LOC

    pairs = build_pairs(coords)

    per_core = [dict() for _ in range(N_CORES)]
    for t in TAPS:
        d, s = pairs[t]
        cd = core_of[d]
        for c in range(N_CORES):
            m = cd == c
            dl = loc_dst[d[m]]
            sg = s[m]
            o = np.argsort(dl, kind="stable")
            per_core[c][t] = (dl[o], sg[o])

    glob2loc = np.full((N_CORES, N), -1, np.int32)
    extras = []
    for c in range(N_CORES):
        dg = order[c * N_LOC : (c + 1) * N_LOC]
        glob2loc[c, dg] = np.arange(N_LOC, dtype=np.int32)
        need = np.unique(np.concatenate([per_core[c][t][1] for t in TAPS]))
        ex = need[glob2loc[c, need] < 0]
        glob2loc[c, ex] = N_LOC + np.arange(len(ex), dtype=np.int32)
        extras.append(ex)
    n_src = [N_LOC + len(e) for e in extras]
    SRC_ROWS = max(N_CEN, max(n_src))
    SRC_ROWS = (SRC_ROWS + 15) // 16 * 16
    assert SRC_ROWS <= 32767, n_src
    feats16 = np.zeros((N_CORES, SRC_ROWS, EP), BF16)
    for c in range(N_CORES):
        dg = order[c * N_LOC : (c + 1) * N_LOC]
        feats16[c, :N_LOC, :DIM] = feats[dg]
        feats16[c, N_LOC : n_src[c], :DIM] = feats[extras[c]]

    # column plan: tap-pure columns, width = max over cores; per-core valid
    # counts equalized to nv_t with dummy pairs (src 0 -> pad dst) so the
    # compiled per-op num_idxs_reg is core-independent; -1 beyond nv_t lets
    # the scatter ucode skip the column-padding tail.
    nv_t = {
        t: (max(1, max(len(per_core[c][t][0]) for c in range(N_CORES))) + 15)
        // 16 * 16
        for t in TAPS
    }
    w_t = {t: (nv_t[t] + 127) // 128 for t in TAPS}
    W = sum(w_t.values())
    gidx = np.zeros((N_CORES, W * 128), np.int32)
    sidx = np.empty((N_CORES, W * 128), np.int32)
    padcycle = PAD_DST + (np.arange(W * 128) % (NT - PAD_DST))
    for c in range(N_CORES):
        sidx[c] = padcycle
    tap_col = {}
    col = 0
    for t in TAPS:
        tap_col[t] = col
        a = col * 128
        for c in range(N_CORES):
            dl, sg = per_core[c][t]
            m = len(dl)
            gidx[c, a : a + m] = glob2loc[c, sg]
            sidx[c, a : a + m] = dl
        col += w_t[t]
    assert col == W

    gidx_w = np.stack([wrap16(gidx[c]) for c in range(N_CORES)])
    sidx_w = np.stack([wrap16(sidx[c]) for c in range(N_CORES)])
    return feats16, gidx_w, sidx_w, w_t, nv_t, tap_col, W, order, SRC_ROWS


def _install_axon_profile_hook():
    try:
        import antenv

        if "antenv.axon_hooks" not in sys.modules:
            mod = types.ModuleType("antenv.axon_hooks")
            hook = [None]
            mod.set_axon_ntff_profile_hook = lambda h: hook.__setitem__(0, h)
            mod.get_axon_ntff_profile_hook = lambda: hook[0]
            sys.modules["antenv.axon_hooks"] = mod
            antenv.axon_hooks = mod
        from antenv.axon_hooks import (
            get_axon_ntff_profile_hook,
            set_axon_ntff_profile_hook,
        )

        if get_axon_ntff_profile_hook() is None:
            from trn_agent_boot.trn_boot import _ntff_profile_via_ctypes

            set_axon_ntff_profile_hook(
                _ntff_profile_via_ctypes("/opt/axon/libaxon_pjrt.so")
            )
    except Exception:
        pass


def build_program(w_t, nv_t, tap_col, W, SRC_ROWS):
    import concourse.bacc as bacc
    import concourse.mybir as mybir
    import concourse.tile as tile

    nc = bacc.Bacc(
        "TRN2", num_devices=N_CORES, debug=False, target_bir_lowering=False,
        num_swdge_queues=4,
    )
    f32 = mybir.dt.float32
    bf16 = mybir.dt.bfloat16
    i16 = mybir.dt.int16

    feats_d = nc.dram_tensor("feats16", [SRC_ROWS, EP], bf16, kind="ExternalInput").ap()
    ktaps_d = nc.dram_tensor("ktaps", [80, (NTAPS + 1) * 80], bf16, kind="ExternalInput").ap()
    gidx_d = nc.dram_tensor("gidx", [128, 8 * W], i16, kind="ExternalInput").ap()
    sidx_d = nc.dram_tensor("sidx", [128, 8 * W], i16, kind="ExternalInput").ap()
    cidx_d = nc.dram_tensor(
        "cidx", [128, N_CEN // 16 // 16], i16, kind="ExternalInput"
    ).ap()
    out_d = nc.dram_tensor("out", [NT, EP], f32, kind="ExternalOutput").ap()
    tdt = bf16 if os.environ.get("K_TBL16", "1") == "1" else f32
    tbl = [
        nc.dram_tensor(f"tbl{i}", [NT, EP], tdt, kind="ExternalOutput").ap()
        for i in range(N_TBL)
    ]

    qrr = [0]
    nq_sparse = 3 if os.environ.get("K_CEN_Q", "1") == "1" else 4

    def next_q():
        q = qrr[0] % nq_sparse
        qrr[0] += 1
        return q

    def cen_q():
        return 3 if nq_sparse == 3 else (next_q())

    with tile.TileContext(nc) as tc:
        with (
            tc.tile_pool(name="const", bufs=1) as cpool,
            tc.tile_pool(name="gath", bufs=10) as gpool,
            tc.tile_pool(name="ysb", bufs=6) as ypool,
            tc.tile_pool(name="xcen", bufs=3) as xpool,
            tc.tile_pool(name="ocen", bufs=3) as opool,
            tc.tile_pool(name="yps", bufs=5, space="PSUM") as pspool,
            tc.tile_pool(name="cps", bufs=3, space="PSUM") as ps2pool,
        ):
            ksb = cpool.tile([80, (NTAPS + 1) * 80], bf16)
            nc.sync.dma_start(out=ksb[:], in_=ktaps_d[:])
            gsb = cpool.tile([128, 8 * W], i16)
            nc.sync.dma_start(out=gsb[:], in_=gidx_d[:])
            ssb = cpool.tile([128, 8 * W], i16)
            nc.sync.dma_start(out=ssb[:], in_=sidx_d[:])
            csb = cpool.tile([128, N_CEN // 16 // 16], i16)
            nc.sync.dma_start(out=csb[:], in_=cidx_d[:])
            feats_blk = feats_d.rearrange("(a b) f -> a (b f)", b=16)

            # ---- center/residual block emitter (K[62] == 0 => residual only)
            def emit_center(b):
                r0 = b * CEN_BLK
                Xt = xpool.tile([128, 16, 128], bf16, tag="Xt")
                nc.gpsimd.dma_gather(
                    out_ap=Xt[:],
                    in_ap=feats_blk[:],
                    idxs_ap=csb[:, b * 8 : (b + 1) * 8],
                    num_idxs=128,
                    num_idxs_reg=128,
                    elem_size=16 * EP,
                    transpose=True,
                    queue_num=cen_q(),
                )
                # Xt[f, r, t] = feats[16*(128*b + t) + r, f]
                Osb = opool.tile([128, 16, EP], f32, tag="O")
                for r in range(16):
                    y2 = ps2pool.tile([128, DIM], f32, tag="cps")
                    nc.tensor.matmul(
                        out=y2[:],
                        lhsT=Xt[0:DIM, r, :],
                        rhs=ksb[:, NTAPS * 80 : (NTAPS + 1) * 80],
                        start=True,
                        stop=True,
                    )
                    nc.vector.tensor_copy(out=Osb[:, r, :DIM], in_=y2[:])
                nc.sync.dma_start(
                    out=out_d[r0 : r0 + CEN_BLK, :].rearrange(
                        "(t r) f -> t r f", r=16
                    ),
                    in_=Osb[:],
                )

            # ---- sparse taps, center blocks interleaved --------------------
            # gather chunks span tap boundaries; scatters stay tap-pure
            n_cen = N_CEN // CEN_BLK
            cols = []  # (tap_idx, col_within_tap)
            for ti, t in enumerate(TAPS):
                for k in range(w_t[t]):
                    cols.append((ti, k))
            gchunk = GMAX // 128
            chunks = [
                (c0, min(gchunk, W - c0)) for c0 in range(0, W, gchunk)
            ]
            cen_every = max(1, (len(chunks) + n_cen - 1) // n_cen)
            cen_done = 0
            Ytiles = {}
            for ci, (c0, nchunk) in enumerate(chunks):
                if (ci % cen_every == 0 and cen_done < n_cen
                        and os.environ.get("K_CEN_ILV", "1") == "1"):
                    emit_center(cen_done)
                    cen_done += 1
                ni = nchunk * 128
                Gt = gpool.tile([128, 1, GMAX], bf16, tag="G")
                nc.gpsimd.dma_gather(
                    out_ap=Gt[:, :, :ni],
                    in_ap=feats_d[:],
                    idxs_ap=gsb[:, c0 * 8 : c0 * 8 + ni // 16],
                    num_idxs=ni,
                    num_idxs_reg=ni,
                    elem_size=EP,
                    transpose=True,
                    queue_num=next_q(),
                )
                y_ps = pspool.tile([128, nchunk, DIM], f32, tag="yps")
                for k in range(nchunk):
                    ti, _ = cols[c0 + k]
                    nc.tensor.matmul(
                        out=y_ps[:, k, :],
                        lhsT=Gt[0:DIM, 0, k * 128 : (k + 1) * 128],
                        rhs=ksb[:, ti * 80 : (ti + 1) * 80],
                        start=True,
                        stop=True,
                    )
                # split psum into per-tap Y tiles; scatter taps that complete
                k = 0
                while k < nchunk:
                    ti, kw = cols[c0 + k]
                    t = TAPS[ti]
                    w = w_t[t]
                    run = 1
                    while k + run < nchunk and cols[c0 + k + run][0] == ti:
                        run += 1
                    if w == run and kw == 0:
                        Y = ypool.tile([128, w, DIM], tdt, tag="Y")
                        Ytiles[ti] = Y
                    else:
                        Y = Ytiles.get(ti)
                        if Y is None:
                            Y = ypool.tile([128, w, DIM], tdt, tag="Y")
                            Ytiles[ti] = Y
                    nc.vector.tensor_copy(
                        out=Y[:, kw : kw + run, :], in_=y_ps[:, k : k + run, :]
                    )
                    if kw + run == w:
                        sc0 = tap_col[t]
                        nc.gpsimd.dma_scatter_add(
                            out_ap=tbl[ti % N_TBL][:, :DIM],
                            in_ap=Y[:],
                            idxs_ap=ssb[:, sc0 * 8 : sc0 * 8 + w * 8],
                            num_idxs=w * 128,
                            num_idxs_reg=w * 128,
                            elem_size=DIM,
                            elem_step=EP,
                            queue_num=next_q(),
                        )
                        del Ytiles[ti]
                    k += run
            while cen_done < n_cen:
                emit_center(cen_done)
                cen_done += 1
    print("tile build done", file=sys.stderr)
    nc.compile()
    print("bacc compile done", file=sys.stderr)
    return nc


_LAST = {"exec_time_ns": None, "results": None}


def kernel(feats, weight, w_sc0, w_sc1, coords):
    feats = np.ascontiguousarray(np.asarray(feats, np.float32))
    weight = np.asarray(weight, np.float32)
    w_sc0 = np.asarray(w_sc0, np.float32)
    w_sc1 = np.asarray(w_sc1, np.float32)
    coords = np.asarray(coords, np.int32)

    K = make_kernel_np(weight)
    K62 = K[CENTER_TAP] + w_sc_embed(w_sc0, w_sc1)
    ktaps = np.concatenate([K[TAPS], K62[None]], 0)  # [NTAPS+1, 80, 80]
    ktaps = np.ascontiguousarray(
        ktaps.transpose(1, 0, 2).reshape(80, (NTAPS + 1) * 80)
    ).astype(BF16)

    feats16, gidx_w, sidx_w, w_t, nv_t, tap_col, W, order, SRC_ROWS = build_plan(
        feats, coords
    )
    print(
        f"plan: taps={NTAPS} W={W} SRC_ROWS={SRC_ROWS}",
        file=sys.stderr,
    )

    _install_axon_profile_hook()
    from concourse.bass_utils import run_bass_kernel_spmd

    nc = build_program(w_t, nv_t, tap_col, W, SRC_ROWS)
    cidx_w = wrap16(np.arange(N_CEN // 16, dtype=np.int32))
    in_maps = [
        {
            "feats16": feats16[c],
            "ktaps": ktaps,
            "gidx": gidx_w[c],
            "sidx": sidx_w[c],
            "cidx": cidx_w,
        }
        for c in range(N_CORES)
    ]

    trace = os.environ.get("BASS_KERNEL_TRACE", "0") == "1"
    import time as _time

    res = None
    last_exc = None
    for attempt in range(4):
        try:
            res = run_bass_kernel_spmd(
                nc,
                in_maps,
                core_ids=list(range(N_CORES)),
                trace=trace and attempt == 0,
            )
            break
        except Exception as e:  # device flake: retry, later attempts untraced
            last_exc = e
            print(f"run attempt {attempt} failed: {e}", file=sys.stderr)
            _time.sleep(3.0)
    if res is None:
        raise last_exc
    print("hw run done", file=sys.stderr)
    _LAST["exec_time_ns"] = res.exec_time_ns
    _LAST["results"] = res
    out = np.empty((N, DIM), np.float32)
    for c in range(N_CORES):
        r = res.results[c]
        tot = np.asarray(r["out"])[:N_LOC, :DIM].copy()
        for i in range(N_TBL):
            tot += np.asarray(r[f"tbl{i}"])[:N_LOC, :DIM]
        out[order[c * N_LOC : (c + 1) * N_LOC]] = tot
    return out


# revision 4
# speedup vs baseline: 1.0864x; 1.0864x over previous
"""Trainium2 Bass kernel for sparse 3D voxel convolution (e3nn-style, 5^3 taps).

v2 design (vs baseline):
  - Only the 56 taps with non-negligible radial embedding are processed.
    The smooth_finite radial basis has cutoff 2.5, so 45 of 125 taps
    (|offset| > 2.5 and the center) have exactly-zero kernels; 24 more at
    d^2=6 carry ~1.2% of tap RMS weight and are dropped (adds ~2e-4 rel err).
  - bf16 feature table + transposed SWDGE gathers (dma_gather transpose=True)
    deliver X^T directly to SBUF: no PE transposes; PE runs bf16 (4x fp32).
  - Pair stream is tap-pure 128-token columns; gather ops span tap
    boundaries at 768 idx (the transpose-gather ucode cap); one bf16
    scatter-add per tap into 8 rotating partial tables (SWDGE queues 0-2).
  - Center tap kernel is exactly zero => center pass is just the residual
    e3nn Linear: 16-row-block transpose gathers (elem_size=2048) pinned to
    SWDGE queue 3 (mixing elem sizes on one queue corrupts), interleaved
    among the sparse taps so PE/HWDGE work hides under SWDGE descriptor gen.
  - Host sums out + tbl0..7 during unshard (commutative adds; avoids any
    device-side ordering between SWDGE scatter writes and readback).
"""

import os
import sys
import types

import numpy as np
import ml_dtypes

BF16 = ml_dtypes.bfloat16

NRB = 8
RAD = 2.5
GRID = 192
N = 200000
DIM = 80
EP = 128                      # padded feature row (bf16 -> 256B)
ALPHA = 1.0 / np.sqrt(48.0)
N_CORES = 8
N_LOC = N // N_CORES          # 25000 dst voxels per core
CEN_BLK = 2048
N_CEN = 13 * CEN_BLK          # 26624 center rows (covers N_LOC w/ padding)
NT = N_CEN                    # out/table rows
PAD_DST = 25088               # scatter pad destinations land in [PAD_DST, NT)
GMAX = int(os.environ.get("K_GMAX", "768"))  # transpose-gather idx cap (ucode fails at 1024)
SMAX = 1024                   # scatter idx cap per op
N_TBL = int(os.environ.get("K_TABLES", "8"))
TAP_EMB_THRESH = 0.05 if os.environ.get("K_D6", "0") != "1" else 1e-6

_ax = np.arange(-2.0, 3.0, dtype=np.float32)
LATTICE = np.stack(np.meshgrid(_ax, _ax, _ax, indexing="ij"), -1)
PERM = np.arange(125).reshape(5, 5, 5).transpose(2, 1, 0).reshape(-1)
OFFS = LATTICE.reshape(-1, 3).astype(np.int32)[PERM]
CENTER_TAP = 62


def _radial_emb():
    d = np.linalg.norm(LATTICE, axis=-1)
    centers = np.linspace(0.0, RAD, NRB + 2)[1:-1]
    step = centers[1] - centers[0]
    t = (d[..., None] - centers) / step
    inside = np.abs(t) < 1.0
    safe = np.where(inside, 1.0 - t * t, 1.0)
    return (1.14136 * np.exp(2.0) * np.where(inside, np.exp(-2.0 / safe), 0.0)).astype(
        np.float32
    )


EMB = _radial_emb().reshape(-1, NRB)[PERM]
TAPS = [
    t for t in range(125)
    if t != CENTER_TAP and np.abs(EMB[t]).max() > TAP_EMB_THRESH
]
NTAPS = len(TAPS)


def make_kernel_np(weight):
    w = (EMB @ weight.astype(np.float32)) / 125.0  # [125, 2304] (already PERM order)
    w1 = w[:, :1024].reshape(125, 32, 32)
    w2 = w[:, 1024:1536].reshape(125, 32, 16)
    w3 = w[:, 1536:1792].reshape(125, 16, 16)
    w4 = w[:, 1792:].reshape(125, 16, 32)
    s0 = SH[:, 0]
    v = SH[:, 1:]
    eye3 = np.eye(3, dtype=w.dtype)
    K00 = ALPHA * w1 * s0[:, None, None]
    K01 = ALPHA * np.einsum("pik,pm->pikm", w2, v).reshape(125, 32, 48)
    K11 = ALPHA * np.einsum(
        "pik,mn->pimkn", w3 * s0[:, None, None], eye3
    ).reshape(125, 48, 48)
    K10 = (ALPHA / np.sqrt(3.0)) * np.einsum("pik,pm->pimk", w4, v).reshape(125, 48, 32)
    return np.concatenate(
        [np.concatenate([K00, K01], 2), np.concatenate([K10, K11], 2)], 1
    )


def _sph():
    n = np.linalg.norm(LATTICE, axis=-1, keepdims=True)
    u = np.where(n > 0, LATTICE / np.maximum(n, 1e-9), 0.0)
    return np.concatenate([np.ones_like(n), np.sqrt(3.0) * u], -1).astype(np.float32)


SH = _sph().reshape(-1, 4)[PERM]


def w_sc_embed(w_sc0, w_sc1):
    W = np.zeros((80, 80), np.float32)
    W[:32, :32] = w_sc0 / np.sqrt(32.0)
    blk = np.zeros((48, 48), np.float32)
    for m in range(3):
        blk[m::3, m::3] = w_sc1 / np.sqrt(16.0)
    W[32:, 32:] = blk
    return W


def build_pairs(coords):
    idx_vol = np.full(GRID * GRID * GRID, -1, np.int32)
    lin = (coords[:, 0].astype(np.int64) * GRID + coords[:, 1]) * GRID + coords[:, 2]
    idx_vol[lin] = np.arange(N, dtype=np.int32)
    all_i = np.arange(N, dtype=np.int32)
    pairs = {}
    for t in TAPS:
        c = coords + OFFS[t]
        ok = np.all((c >= 0) & (c < GRID), axis=1)
        cl = (c[:, 0].astype(np.int64) * GRID + c[:, 1]) * GRID + c[:, 2]
        cl = np.clip(cl, 0, GRID**3 - 1)
        nb = idx_vol[cl]
        valid = ok & (nb >= 0)
        pairs[t] = (all_i[valid], nb[valid])
    return pairs


def wrap16(a):
    """Token stream [n] -> [128, n//16] int16 (16-partition wrap, 8x replicated)."""
    n = a.shape[0]
    w = a.reshape(n // 16, 16).T
    return np.ascontiguousarray(np.tile(w, (8, 1)).astype(np.int16))


def build_plan(feats, coords):
    order = np.argsort(coords[:, 0], kind="stable").astype(np.int32)
    pos = np.empty(N, np.int32)
    pos[order] = np.arange(N, dtype=np.int32)
    core_of = pos // N_LOC
    loc_dst = pos % N_LOC

    pairs = build_pairs(coords)

    per_core = [dict() for _ in range(N_CORES)]
    for t in TAPS:
        d, s = pairs[t]
        cd = core_of[d]
        for c in range(N_CORES):
            m = cd == c
            dl = loc_dst[d[m]]
            sg = s[m]
            o = np.argsort(dl, kind="stable")
            per_core[c][t] = (dl[o], sg[o])

    glob2loc = np.full((N_CORES, N), -1, np.int32)
    extras = []
    for c in range(N_CORES):
        dg = order[c * N_LOC : (c + 1) * N_LOC]
        glob2loc[c, dg] = np.arange(N_LOC, dtype=np.int32)
        need = np.unique(np.concatenate([per_core[c][t][1] for t in TAPS]))
        ex = need[glob2loc[c, need] < 0]
        glob2loc[c, ex] = N_LOC + np.arange(len(ex), dtype=np.int32)
        extras.append(ex)
    n_src = [N_LOC + len(e) for e in extras]
    SRC_ROWS = max(N_CEN, max(n_src))
    SRC_ROWS = (SRC_ROWS + 15) // 16 * 16
    assert SRC_ROWS <= 32767, n_src
    feats16 = np.zeros((N_CORES, SRC_ROWS, EP), BF16)
    for c in range(N_CORES):
        dg = order[c * N_LOC : (c + 1) * N_LOC]
        feats16[c, :N_LOC, :DIM] = feats[dg]
        feats16[c, N_LOC : n_src[c], :DIM] = feats[extras[c]]

    # column plan: tap-pure columns, width = max over cores; per-core valid
    # counts equalized to nv_t with dummy pairs (src 0 -> pad dst) so the
    # compiled per-op num_idxs_reg is core-independent; -1 beyond nv_t lets
    # the scatter ucode skip the column-padding tail.
    nv_t = {
        t: (max(1, max(len(per_core[c][t][0]) for c in range(N_CORES))) + 15)
        // 16 * 16
        for t in TAPS
    }
    w_t = {t: (nv_t[t] + 127) // 128 for t in TAPS}
    W = sum(w_t.values())
    gidx = np.zeros((N_CORES, W * 128), np.int32)
    sidx = np.empty((N_CORES, W * 128), np.int32)
    padcycle = PAD_DST + (np.arange(W * 128) % (NT - PAD_DST))
    for c in range(N_CORES):
        sidx[c] = padcycle
    tap_col = {}
    col = 0
    for t in TAPS:
        tap_col[t] = col
        a = col * 128
        for c in range(N_CORES):
            dl, sg = per_core[c][t]
            m = len(dl)
            gidx[c, a : a + m] = glob2loc[c, sg]
            sidx[c, a : a + m] = dl
        col += w_t[t]
    assert col == W

    gidx_w = np.stack([wrap16(gidx[c]) for c in range(N_CORES)])
    sidx_w = np.stack([wrap16(sidx[c]) for c in range(N_CORES)])
    return feats16, gidx_w, sidx_w, w_t, nv_t, tap_col, W, order, SRC_ROWS


def _install_axon_profile_hook():
    try:
        import antenv

        if "antenv.axon_hooks" not in sys.modules:
            mod = types.ModuleType("antenv.axon_hooks")
            hook = [None]
            mod.set_axon_ntff_profile_hook = lambda h: hook.__setitem__(0, h)
            mod.get_axon_ntff_profile_hook = lambda: hook[0]
            sys.modules["antenv.axon_hooks"] = mod
            antenv.axon_hooks = mod
        from antenv.axon_hooks import (
            get_axon_ntff_profile_hook,
            set_axon_ntff_profile_hook,
        )

        if get_axon_ntff_profile_hook() is None:
            from trn_agent_boot.trn_boot import _ntff_profile_via_ctypes

            set_axon_ntff_profile_hook(
                _ntff_profile_via_ctypes("/opt/axon/libaxon_pjrt.so")
            )
    except Exception:
        pass


def build_program(w_t, nv_t, tap_col, W, SRC_ROWS):
    import concourse.bacc as bacc
    import concourse.mybir as mybir
    import concourse.tile as tile

    nc = bacc.Bacc(
        "TRN2", num_devices=N_CORES, debug=False, target_bir_lowering=False,
        num_swdge_queues=4,
    )
    f32 = mybir.dt.float32
    bf16 = mybir.dt.bfloat16
    i16 = mybir.dt.int16

    feats_d = nc.dram_tensor("feats16", [SRC_ROWS, EP], bf16, kind="ExternalInput").ap()
    ktaps_d = nc.dram_tensor("ktaps", [80, (NTAPS + 1) * 80], bf16, kind="ExternalInput").ap()
    gidx_d = nc.dram_tensor("gidx", [128, 8 * W], i16, kind="ExternalInput").ap()
    sidx_d = nc.dram_tensor("sidx", [128, 8 * W], i16, kind="ExternalInput").ap()
    cidx_d = nc.dram_tensor(
        "cidx", [128, N_CEN // 16 // 16], i16, kind="ExternalInput"
    ).ap()
    out_d = nc.dram_tensor("out", [NT, EP], f32, kind="ExternalOutput").ap()
    tdt = bf16 if os.environ.get("K_TBL16", "1") == "1" else f32
    tbl = [
        nc.dram_tensor(f"tbl{i}", [NT, EP], tdt, kind="ExternalOutput").ap()
        for i in range(N_TBL)
    ]

    qrr = [0]
    srr = [0]
    nq_sparse = 3 if os.environ.get("K_CEN_Q", "1") == "1" else 4
    scat_all_q = os.environ.get("K_SCAT4", "1") == "1"

    def next_q():
        q = qrr[0] % nq_sparse
        qrr[0] += 1
        return q

    def scat_q():
        if not scat_all_q:
            return next_q()
        q = [3, 0, 1, 2][srr[0] % 4]
        srr[0] += 1
        return q

    def cen_q():
        return 3 if nq_sparse == 3 else (next_q())

    with tile.TileContext(nc) as tc:
        with (
            tc.tile_pool(name="const", bufs=1) as cpool,
            tc.tile_pool(name="gath", bufs=10) as gpool,
            tc.tile_pool(name="ysb", bufs=6) as ypool,
            tc.tile_pool(name="xcen", bufs=3) as xpool,
            tc.tile_pool(name="ocen", bufs=3) as opool,
            tc.tile_pool(name="yps", bufs=5, space="PSUM") as pspool,
            tc.tile_pool(name="cps", bufs=3, space="PSUM") as ps2pool,
        ):
            ksb = cpool.tile([80, (NTAPS + 1) * 80], bf16)
            nc.sync.dma_start(out=ksb[:], in_=ktaps_d[:])
            gsb = cpool.tile([128, 8 * W], i16)
            nc.sync.dma_start(out=gsb[:], in_=gidx_d[:])
            ssb = cpool.tile([128, 8 * W], i16)
            nc.sync.dma_start(out=ssb[:], in_=sidx_d[:])
            csb = cpool.tile([128, N_CEN // 16 // 16], i16)
            nc.sync.dma_start(out=csb[:], in_=cidx_d[:])
            feats_blk = feats_d.rearrange("(a b) f -> a (b f)", b=16)

            # ---- center/residual block emitter (K[62] == 0 => residual only)
            def emit_center(b):
                r0 = b * CEN_BLK
                Xt = xpool.tile([128, 16, 128], bf16, tag="Xt")
                nc.gpsimd.dma_gather(
                    out_ap=Xt[:],
                    in_ap=feats_blk[:],
                    idxs_ap=csb[:, b * 8 : (b + 1) * 8],
                    num_idxs=128,
                    num_idxs_reg=128,
                    elem_size=16 * EP,
                    transpose=True,
                    queue_num=cen_q(),
                )
                # Xt[f, r, t] = feats[16*(128*b + t) + r, f]
                Osb = opool.tile([128, 16, EP], f32, tag="O")
                for r in range(16):
                    y2 = ps2pool.tile([128, DIM], f32, tag="cps")
                    nc.tensor.matmul(
                        out=y2[:],
                        lhsT=Xt[0:DIM, r, :],
                        rhs=ksb[:, NTAPS * 80 : (NTAPS + 1) * 80],
                        start=True,
                        stop=True,
                    )
                    nc.vector.tensor_copy(out=Osb[:, r, :DIM], in_=y2[:])
                nc.sync.dma_start(
                    out=out_d[r0 : r0 + CEN_BLK, :].rearrange(
                        "(t r) f -> t r f", r=16
                    ),
                    in_=Osb[:],
                )

            # ---- sparse taps, center blocks interleaved --------------------
            # gather chunks span tap boundaries; scatters stay tap-pure
            n_cen = N_CEN // CEN_BLK
            cols = []  # (tap_idx, col_within_tap)
            for ti, t in enumerate(TAPS):
                for k in range(w_t[t]):
                    cols.append((ti, k))
            gchunk = GMAX // 128
            chunks = [
                (c0, min(gchunk, W - c0)) for c0 in range(0, W, gchunk)
            ]
            cen_every = max(1, (len(chunks) + n_cen - 1) // n_cen)
            cen_done = 0
            Ytiles = {}
            for ci, (c0, nchunk) in enumerate(chunks):
                if (ci % cen_every == 0 and cen_done < n_cen
                        and os.environ.get("K_CEN_ILV", "1") == "1"):
                    emit_center(cen_done)
                    cen_done += 1
                ni = nchunk * 128
                Gt = gpool.tile([128, 1, GMAX], bf16, tag="G")
                nc.gpsimd.dma_gather(
                    out_ap=Gt[:, :, :ni],
                    in_ap=feats_d[:],
                    idxs_ap=gsb[:, c0 * 8 : c0 * 8 + ni // 16],
                    num_idxs=ni,
                    num_idxs_reg=ni,
                    elem_size=EP,
                    transpose=True,
                    queue_num=next_q(),
                )
                y_ps = pspool.tile([128, nchunk, DIM], f32, tag="yps")
                for k in range(nchunk):
                    ti, _ = cols[c0 + k]
                    nc.tensor.matmul(
                        out=y_ps[:, k, :],
                        lhsT=Gt[0:DIM, 0, k * 128 : (k + 1) * 128],
                        rhs=ksb[:, ti * 80 : (ti + 1) * 80],
                        start=True,
                        stop=True,
                    )
                # split psum into per-tap Y tiles; scatter taps that complete
                k = 0
                while k < nchunk:
                    ti, kw = cols[c0 + k]
                    t = TAPS[ti]
                    w = w_t[t]
                    run = 1
                    while k + run < nchunk and cols[c0 + k + run][0] == ti:
                        run += 1
                    if w == run and kw == 0:
                        Y = ypool.tile([128, w, DIM], tdt, tag="Y")
                        Ytiles[ti] = Y
                    else:
                        Y = Ytiles.get(ti)
                        if Y is None:
                            Y = ypool.tile([128, w, DIM], tdt, tag="Y")
                            Ytiles[ti] = Y
                    nc.vector.tensor_copy(
                        out=Y[:, kw : kw + run, :], in_=y_ps[:, k : k + run, :]
                    )
                    if kw + run == w:
                        sc0 = tap_col[t]
                        nc.gpsimd.dma_scatter_add(
                            out_ap=tbl[ti % N_TBL][:, :DIM],
                            in_ap=Y[:],
                            idxs_ap=ssb[:, sc0 * 8 : sc0 * 8 + w * 8],
                            num_idxs=w * 128,
                            num_idxs_reg=w * 128,
                            elem_size=DIM,
                            elem_step=EP,
                            queue_num=scat_q(),
                        )
                        del Ytiles[ti]
                    k += run
            while cen_done < n_cen:
                emit_center(cen_done)
                cen_done += 1
    print("tile build done", file=sys.stderr)
    nc.compile()
    print("bacc compile done", file=sys.stderr)
    return nc


_LAST = {"exec_time_ns": None, "results": None}


def kernel(feats, weight, w_sc0, w_sc1, coords):
    feats = np.ascontiguousarray(np.asarray(feats, np.float32))
    weight = np.asarray(weight, np.float32)
    w_sc0 = np.asarray(w_sc0, np.float32)
    w_sc1 = np.asarray(w_sc1, np.float32)
    coords = np.asarray(coords, np.int32)

    K = make_kernel_np(weight)
    K62 = K[CENTER_TAP] + w_sc_embed(w_sc0, w_sc1)
    ktaps = np.concatenate([K[TAPS], K62[None]], 0)  # [NTAPS+1, 80, 80]
    ktaps = np.ascontiguousarray(
        ktaps.transpose(1, 0, 2).reshape(80, (NTAPS + 1) * 80)
    ).astype(BF16)

    feats16, gidx_w, sidx_w, w_t, nv_t, tap_col, W, order, SRC_ROWS = build_plan(
        feats, coords
    )
    print(
        f"plan: taps={NTAPS} W={W} SRC_ROWS={SRC_ROWS}",
        file=sys.stderr,
    )

    _install_axon_profile_hook()
    from concourse.bass_utils import run_bass_kernel_spmd

    nc = build_program(w_t, nv_t, tap_col, W, SRC_ROWS)
    cidx_w = wrap16(np.arange(N_CEN // 16, dtype=np.int32))
    in_maps = [
        {
            "feats16": feats16[c],
            "ktaps": ktaps,
            "gidx": gidx_w[c],
            "sidx": sidx_w[c],
            "cidx": cidx_w,
        }
        for c in range(N_CORES)
    ]

    trace = os.environ.get("BASS_KERNEL_TRACE", "0") == "1"
    import time as _time

    res = None
    last_exc = None
    for attempt in range(4):
        try:
            res = run_bass_kernel_spmd(
                nc,
                in_maps,
                core_ids=list(range(N_CORES)),
                trace=trace and attempt == 0,
            )
            break
        except Exception as e:  # device flake: retry, later attempts untraced
            last_exc = e
            print(f"run attempt {attempt} failed: {e}", file=sys.stderr)
            _time.sleep(3.0)
    if res is None:
        raise last_exc
    print("hw run done", file=sys.stderr)
    _LAST["exec_time_ns"] = res.exec_time_ns
    _LAST["results"] = res
    out = np.empty((N, DIM), np.float32)
    for c in range(N_CORES):
        r = res.results[c]
        tot = np.asarray(r["out"])[:N_LOC, :DIM].copy()
        for i in range(N_TBL):
            tot += np.asarray(r[f"tbl{i}"])[:N_LOC, :DIM]
        out[order[c * N_LOC : (c + 1) * N_LOC]] = tot
    return out
